# revision 35
# baseline (speedup 1.0000x reference)
import sys

sys.path.insert(0, "/opt/trn_rl_repo")

import numpy as np
import ml_dtypes

import concourse.bass as bass
import concourse.tile as tile
from concourse import bacc, mybir
from concourse.bass_utils import run_bass_kernel_spmd

# Problem constants (hardcoded per contract)
B, N, F = 8, 512, 16
D, PH, PW = 150, 26, 26
IMG = 128
C = 64  # spline coefficients per voxel
HW = PH * PW  # 676
NG = 132  # groups of 4 emitters; each group contracts K=128 = 2 stacked slabs
SLOTS = 4  # emitters per group; each slot owns 32 partitions (16 real + pad)
SW = 32  # slot width in partitions (engine partition bases must be 32-aligned)
ROW0 = 14  # min y_idx / x_idx given input coordinate ranges
CSTRIDE = 152  # canvas row stride (rows/cols span [14, 165) -> 151 + spill)
CV_ELEMS = 152 * CSTRIDE  # canvas elems per partition (incl. ds-claim spill)
OFF_MAX = (139 - ROW0) * CSTRIDE + (139 - ROW0)  # largest scatter offset
# The DVE canvas is doubled: even-x patches go to region A at their offset,
# odd-x patches to region B at offset CV_ELEMS + off - 1 (even!), so every
# DVE scatter is 4B-aligned and runs in the bf16 2x DVE mode.  Region B's
# columns are shifted by one; the host shifts the crop back when summing.
OFF_MAX_V = CV_ELEMS + OFF_MAX - 1

# Engine map: slot 0 on GPSIMD (GPSIMD ops must have ALL operands at
# partition base 0 -- other bases hard-crash the runtime).  Slots 1-3 on
# DVE, whose datapath handles in1 at partition bases 32/64/96 with out at
# base 0 (dynamic-offset ops bypass the same-base verifier rule and work).
USE_GP_SCATTER = True


def _gp_slot(g, i):
    if not USE_GP_SCATTER:
        return False
    return i == 0


N_GP = sum(_gp_slot(g, i) for g in range(NG) for i in range(SLOTS))
N_DVE = NG * SLOTS - N_GP
_compiled = None


def _build_bass():
    nc = bacc.Bacc()
    f32 = mybir.dt.float32
    bf16 = mybir.dt.bfloat16
    i32 = mybir.dt.int32

    # packed per-group buffer: [:, 0:128] lhsT, [:, 128:804] rhs slabs
    grp_d = nc.declare_dram_parameter("grp", [NG, 128, 128 + HW], bf16, isOutput=False)
    offs_v_d = nc.declare_dram_parameter("offs_v", [1, N_DVE + 1], i32, isOutput=False)
    offs_g_d = nc.declare_dram_parameter("offs_g", [1, N_GP + 1], i32, isOutput=False)
    # canvas crops DMA'd out separately (DVE even, DVE odd-shifted, GP);
    # host shifts/sums them
    out_d = nc.declare_dram_parameter("out", [3, F, IMG * IMG], bf16, isOutput=True)

    with tile.TileContext(nc) as tc:
        with (
            tc.tile_pool(name="canvas", bufs=1) as canvas_pool,
            tc.tile_pool(name="weights", bufs=4) as w_pool,
            tc.tile_pool(name="slabs", bufs=4) as s_pool,
            tc.tile_pool(name="psum", bufs=3, space="PSUM") as p_pool,
            tc.tile_pool(name="small", bufs=1) as small_pool,
        ):
            # two canvas tiles, both at partition base 0 (DVE one doubled)
            canvas_v = canvas_pool.tile([SW, 2 * CV_ELEMS], bf16, tag="cv")
            canvas_g = canvas_pool.tile([SW, CV_ELEMS], bf16, tag="cg")
            nc.scalar.memzero(canvas_v[:])
            nc.scalar.memzero(canvas_g[:])

            offs_v_t = small_pool.tile([1, N_DVE + 1], i32)
            offs_g_t = small_pool.tile([1, N_GP + 1], i32)
            nc.sync.dma_start(offs_v_t[:], offs_v_d[:])
            nc.sync.dma_start(offs_g_t[:], offs_g_d[:])
            preg_v = nc.vector.alloc_register64("offp_v")
            preg_g = nc.gpsimd.alloc_register64("offp_g")

            nv = ng = 0
            for g in range(NG):
                gt = s_pool.tile([128, 128 + HW], bf16, tag="gt")
                nc.sync.dma_start(gt[:], grp_d[g])
                lt = gt[:, 0:128]
                rt = gt[:, 128 : 128 + HW]
                ps = p_pool.tile([128, HW], f32, tag="ps")
                for n0, n1 in ((0, 512), (512, HW)):
                    nc.tensor.matmul(
                        ps[:, n0:n1],
                        lhsT=lt,
                        rhs=rt[:, n0:n1],
                        start=True,
                        stop=True,
                    )
                sbp = s_pool.tile([128, HW], bf16, tag="sbp")
                nc.scalar.copy(out=sbp[:], in_=ps[:])
                ps3 = sbp[:].rearrange("p (h w) -> p h w", h=PH, w=PW)
                for i in range(SLOTS):
                    if _gp_slot(g, i):
                        eng, preg, offs_t, k = nc.gpsimd, preg_g, offs_g_t, ng
                        cnv = canvas_g  # slot 0: all operands at base 0
                        ng += 1
                    else:
                        eng, preg, offs_t, k = nc.vector, preg_v, offs_v_t, nv
                        cnv = canvas_v
                        nv += 1
                    if k % 2 == 0:
                        eng.reg_load(preg, offs_t[0:1, k : k + 2])
                    off = eng.snap(
                        preg.lo if k % 2 == 0 else preg.hi,
                        donate=True,
                        min_val=0,
                        max_val=OFF_MAX if _gp_slot(g, i) else OFF_MAX_V,
                    )
                    dst = (
                        cnv[:, bass.ds(off, PH * CSTRIDE)]
                        .rearrange("p (h w) -> p h w", h=PH)[:, :, 0:PW]
                    )
                    eng.tensor_tensor(
                        out=dst,
                        in0=dst,
                        in1=ps3[SW * i : SW * (i + 1)],
                        op=mybir.AluOpType.add,
                    )

            # DMA canvas crops out; host shifts/sums the three.
            # crop: rows/cols [26, 154) -> canvas-local [12, 140); region B
            # (odd-x patches) is shifted one column left in canvas space.
            crop_off = 12 * CSTRIDE + 12
            for j, (cnv, c0) in enumerate(
                (
                    (canvas_v, crop_off),
                    (canvas_v, CV_ELEMS + crop_off - 1),
                    (canvas_g, crop_off),
                )
            ):
                cb = cnv[0:F, c0 : c0 + IMG * CSTRIDE].rearrange(
                    "p (h w) -> p h w", w=CSTRIDE
                )[:, :, 0:IMG]
                nc.sync.dma_start(
                    out_d[j].rearrange("p (h w) -> p h w", h=IMG, w=IMG),
                    cb,
                )
    if not nc.is_finalized():
        nc.finalize()
    return nc


def _pack_bins(z_idx):
    """Pack 512 emitters into bins of SLOTS slots, each bin drawing from at
    most 2 distinct z-buckets.  Buckets are grouped into units (singletons or
    pairs whose remainders mod SLOTS sum to <= SLOTS); each unit is laid out
    sequentially so every bin touches at most 2 buckets.  Returns list of NG
    bins: (emitter_idx_list, half_list, zA, zB); short bins = dead slots."""
    import collections

    buckets = collections.defaultdict(list)
    for e, z in enumerate(z_idx):
        buckets[int(z)].append(e)
    items = list(buckets.items())
    units = [[it] for it in items if len(it[1]) % SLOTS == 0]
    rn = sorted(
        (it for it in items if len(it[1]) % SLOTS != 0),
        key=lambda t: len(t[1]) % SLOTS,
    )
    lo, hi = 0, len(rn) - 1
    while lo < hi:
        if (len(rn[lo][1]) % SLOTS) + (len(rn[hi][1]) % SLOTS) <= SLOTS:
            units.append([rn[lo], rn[hi]])
            lo += 1
            hi -= 1
        else:
            units.append([rn[hi]])
            hi -= 1
    if lo == hi:
        units.append([rn[lo]])

    bins = []
    for unit in units:
        stream = [(z, e) for z, es in unit for e in es]
        for s0 in range(0, len(stream), SLOTS):
            chunk = stream[s0 : s0 + SLOTS]
            zs = []
            for z, _ in chunk:
                if z not in zs:
                    zs.append(z)
            assert len(zs) <= 2
            zA = zs[0]
            zB = zs[1] if len(zs) > 1 else zs[0]
            binE = [e for _, e in chunk]
            half = [0 if z == zA else 1 for z, _ in chunk]
            bins.append((binE, half, zA, zB))
    assert len(bins) <= NG, f"packing produced {len(bins)} bins > {NG}"
    while len(bins) < NG:
        bins.append(([], [], 0, 0))
    return bins


def _host_prep(xyz, n_photons, coeffs, inv_voxel_size, psf_center):
    u = xyz * inv_voxel_size  # (B,N,3)
    u = u.copy()
    u[..., :2] -= psf_center[:2]
    u[..., 2] += psf_center[2]
    u_floor = np.floor(u)
    frac = u - u_floor
    ui = u_floor.astype(np.int32)
    x_idx = ui[..., 0] + PW  # (B,N)
    y_idx = ui[..., 1] + PH
    z_idx = ui[..., 2]
    frac[..., :2] = 1.0 - frac[..., :2]

    # 64-term series: series[b,n,c], c = kz*16 + kx*4 + ky
    p = frac[..., None] ** np.arange(4, dtype=np.float32)  # (B,N,3,4)
    vx, vy, vz = p[..., 0, :], p[..., 1, :], p[..., 2, :]
    series = (
        vz[..., :, None, None] * vx[..., None, :, None] * vy[..., None, None, :]
    ).reshape(B, N, C)
    s16 = n_photons[..., None] * series[:, :, None, :]  # (B,N,F,C)

    coeffs_t = np.ascontiguousarray(
        coeffs.reshape(D, HW, C).transpose(0, 2, 1)
    ).astype(ml_dtypes.bfloat16)  # (D, C, HW)

    off_all = ((y_idx - ROW0) * CSTRIDE + (x_idx - ROW0)).astype(np.int32)

    grp = np.zeros((B, NG, 128, 128 + HW), dtype=ml_dtypes.bfloat16)
    lhsT = np.zeros((128, 128), dtype=np.float32)
    offs_v = np.zeros((B, 1, N_DVE + 1), dtype=np.int32)
    offs_g = np.zeros((B, 1, N_GP + 1), dtype=np.int32)

    for b in range(B):
        bins = _pack_bins(z_idx[b])
        nv = ng = 0
        for g, (binE, half, zA, zB) in enumerate(bins):
            grp[b, g, 0:64, 128:] = coeffs_t[zA]
            grp[b, g, 64:128, 128:] = coeffs_t[zB]
            # steer an even-x emitter into slot 0: GPSIMD ops then stay
            # 4B-aligned (DVE handles any parity via its doubled canvas).
            # Swap only within the zA block (half assignment preserved).
            if USE_GP_SCATTER and len(binE) > 1:
                for j in range(len(binE)):
                    if half[j] != 0:
                        break
                    if x_idx[b, binE[j]] % 2 == 0:
                        binE[0], binE[j] = binE[j], binE[0]
                        break
            lhsT[:] = 0.0
            for i in range(SLOTS):
                if i < len(binE):
                    e, h = binE[i], half[i]
                    lhsT[h * 64 : h * 64 + 64, i * SW : i * SW + F] = s16[b, e].T
                    off = int(off_all[b, e])
                    odd = (x_idx[b, e] - ROW0) % 2
                else:
                    off = 0  # dead slot: zero weights, scatter adds zeros
                    odd = 0
                if _gp_slot(g, i):
                    offs_g[b, 0, ng] = off
                    ng += 1
                else:
                    # odd-x -> doubled-canvas region B at an even offset
                    offs_v[b, 0, nv] = off if not odd else CV_ELEMS + off - 1
                    nv += 1
            grp[b, g, :, 0:128] = lhsT
        assert nv == N_DVE and ng == N_GP

    return grp, offs_v, offs_g


def make_in_maps(np_inputs):
    grp, offs_v, offs_g = _host_prep(
        np.asarray(np_inputs["xyz"], dtype=np.float32),
        np.asarray(np_inputs["n_photons"], dtype=np.float32),
        np.asarray(np_inputs["coeffs"], dtype=np.float32),
        np.asarray(np_inputs["inv_voxel_size"], dtype=np.float32),
        np.asarray(np_inputs["psf_center"], dtype=np.float32),
    )
    return [
        {
            "grp": grp[b],
            "offs_v": offs_v[b],
            "offs_g": offs_g[b],
        }
        for b in range(B)
    ]


def get_compiled():
    global _compiled
    if _compiled is None:
        _compiled = _build_bass()
    return _compiled


def kernel(xyz, n_photons, coeffs, inv_voxel_size, psf_center, img_size):
    in_maps = make_in_maps(
        {
            "xyz": xyz,
            "n_photons": n_photons,
            "coeffs": coeffs,
            "inv_voxel_size": inv_voxel_size,
            "psf_center": psf_center,
        }
    )
    nc = get_compiled()
    res = run_bass_kernel_spmd(nc, in_maps, core_ids=list(range(B)))
    out = np.stack(
        [
            res.results[b]["out"]
            .astype(np.float32)
            .reshape(3, F, IMG, IMG)
            .sum(axis=0)
            for b in range(B)
        ],
        axis=0,
    )
    return out


# revision 41
# speedup vs baseline: 1.0313x; 1.0313x over previous
import sys

sys.path.insert(0, "/opt/trn_rl_repo")

import numpy as np
import ml_dtypes

import concourse.bass as bass
import concourse.tile as tile
from concourse import bacc, mybir
from concourse.bass_utils import run_bass_kernel_spmd

# Problem constants (hardcoded per contract)
B, N, F = 8, 512, 16
D, PH, PW = 150, 26, 26
IMG = 128
C = 64  # spline coefficients per voxel
HW = PH * PW  # 676
NG = 131  # groups of 4 emitters; each group contracts K=128 = 2 stacked slabs
SLOTS = 4  # emitters per group; each slot owns 32 partitions (16 real + pad)
SW = 32  # slot width in partitions (engine partition bases must be 32-aligned)
ROW0 = 14  # min y_idx / x_idx given input coordinate ranges
CSTRIDE = 152  # canvas row stride (rows/cols span [14, 165) -> 151 + spill)
CV_ELEMS = 152 * CSTRIDE  # canvas elems per partition (incl. ds-claim spill)
OFF_MAX = (139 - ROW0) * CSTRIDE + (139 - ROW0)  # largest scatter offset
# The DVE canvas is doubled: even-x patches go to region A at their offset,
# odd-x patches to region B at offset CV_ELEMS + off - 1 (even!), so every
# DVE scatter is 4B-aligned and runs in the bf16 2x DVE mode.  Region B's
# columns are shifted by one; the host shifts the crop back when summing.
OFF_MAX_V = CV_ELEMS + OFF_MAX - 1

# Engine map: slot 0 on GPSIMD (GPSIMD ops must have ALL operands at
# partition base 0 -- other bases hard-crash the runtime).  Slots 1-3 on
# DVE, whose datapath handles in1 at partition bases 32/64/96 with out at
# base 0 (dynamic-offset ops bypass the same-base verifier rule and work).
USE_GP_SCATTER = True


def _gp_slot(g, i):
    if not USE_GP_SCATTER:
        return False
    return i == 0


N_GP = sum(_gp_slot(g, i) for g in range(NG) for i in range(SLOTS))
N_DVE = NG * SLOTS - N_GP
_compiled = None


def _build_bass():
    nc = bacc.Bacc()
    f32 = mybir.dt.float32
    bf16 = mybir.dt.bfloat16
    i32 = mybir.dt.int32

    # packed per-group buffer: [:, 0:128] lhsT, [:, 128:804] rhs slabs
    grp_d = nc.declare_dram_parameter("grp", [NG, 128, 128 + HW], bf16, isOutput=False)
    offs_v_d = nc.declare_dram_parameter("offs_v", [1, N_DVE + 1], i32, isOutput=False)
    offs_g_d = nc.declare_dram_parameter("offs_g", [1, N_GP + 1], i32, isOutput=False)
    # canvas crops DMA'd out separately (DVE even, DVE odd-shifted, GP);
    # host shifts/sums them
    out_d = nc.declare_dram_parameter("out", [3, F, IMG * IMG], bf16, isOutput=True)

    with tile.TileContext(nc) as tc:
        with (
            tc.tile_pool(name="canvas", bufs=1) as canvas_pool,
            tc.tile_pool(name="weights", bufs=4) as w_pool,
            tc.tile_pool(name="slabs", bufs=4) as s_pool,
            tc.tile_pool(name="psum", bufs=4, space="PSUM") as p_pool,
            tc.tile_pool(name="small", bufs=1) as small_pool,
        ):
            # two canvas tiles, both at partition base 0 (DVE one doubled;
            # DVE ops' in1 reads shift down from bases 32/64/96 to 0 -- the
            # only cross-partition-base pattern the hardware accepts)
            canvas_va = canvas_pool.tile([SW, 2 * CV_ELEMS], bf16, tag="cv")
            canvas_v = canvas_va[0:SW]
            canvas_g = canvas_pool.tile([SW, CV_ELEMS], bf16, tag="cg")
            nc.scalar.memzero(canvas_v)
            nc.gpsimd.memset(canvas_g[:], 0.0)  # parallel with scalar's zero

            offs_v_t = small_pool.tile([1, N_DVE + 1], i32)
            offs_g_t = small_pool.tile([1, N_GP + 1], i32)
            nc.sync.dma_start(offs_v_t[:], offs_v_d[:])
            nc.sync.dma_start(offs_g_t[:], offs_g_d[:])
            preg_v = nc.vector.alloc_register64("offp_v")
            preg_g = nc.gpsimd.alloc_register64("offp_g")

            nv = ng = 0
            for g in range(NG):
                gt = s_pool.tile([128, 128 + HW], bf16, tag="gt")
                nc.sync.dma_start(gt[:], grp_d[g])
                lt = gt[:, 0:128]
                rt = gt[:, 128 : 128 + HW]
                ps = p_pool.tile([128, HW], f32, tag="ps")
                for n0, n1 in ((0, 512), (512, HW)):
                    nc.tensor.matmul(
                        ps[:, n0:n1],
                        lhsT=lt,
                        rhs=rt[:, n0:n1],
                        start=True,
                        stop=True,
                    )
                sbp = s_pool.tile([128, HW], bf16, tag="sbp")
                nc.scalar.copy(out=sbp[:], in_=ps[:])
                ps3 = sbp[:].rearrange("p (h w) -> p h w", h=PH, w=PW)
                for i in range(SLOTS):
                    if _gp_slot(g, i):
                        eng, preg, offs_t, k = nc.gpsimd, preg_g, offs_g_t, ng
                        cnv = canvas_g  # slot 0: all operands at base 0
                        ng += 1
                    else:
                        eng, preg, offs_t, k = nc.vector, preg_v, offs_v_t, nv
                        cnv = canvas_v
                        nv += 1
                    if k % 2 == 0:
                        eng.reg_load(preg, offs_t[0:1, k : k + 2])
                    off = eng.snap(
                        preg.lo if k % 2 == 0 else preg.hi,
                        donate=True,
                        min_val=0,
                        max_val=OFF_MAX if _gp_slot(g, i) else OFF_MAX_V,
                    )
                    dst = (
                        cnv[:, bass.ds(off, PH * CSTRIDE)]
                        .rearrange("p (h w) -> p h w", h=PH)[:, :, 0:PW]
                    )
                    eng.tensor_tensor(
                        out=dst,
                        in0=dst,
                        in1=ps3[SW * i : SW * (i + 1)],
                        op=mybir.AluOpType.add,
                    )

            # DMA canvas crops out; host shifts/sums the three.
            # crop: rows/cols [26, 154) -> canvas-local [12, 140); region B
            # (odd-x patches) is shifted one column left in canvas space.
            crop_off = 12 * CSTRIDE + 12
            for j, (cnv, c0) in enumerate(
                (
                    (canvas_v, crop_off),
                    (canvas_v, CV_ELEMS + crop_off - 1),
                    (canvas_g, crop_off),
                )
            ):
                cb = cnv[0:F, c0 : c0 + IMG * CSTRIDE].rearrange(
                    "p (h w) -> p h w", w=CSTRIDE
                )[:, :, 0:IMG]
                nc.sync.dma_start(
                    out_d[j].rearrange("p (h w) -> p h w", h=IMG, w=IMG),
                    cb,
                )
    if not nc.is_finalized():
        nc.finalize()
    return nc


def _pack_bins(z_idx):
    """Pack 512 emitters into bins of SLOTS slots, each bin drawing from at
    most 2 distinct z-buckets.  Buckets are grouped into units (singletons or
    pairs whose remainders mod SLOTS sum to <= SLOTS); each unit is laid out
    sequentially so every bin touches at most 2 buckets.  Returns list of NG
    bins: (emitter_idx_list, half_list, zA, zB); short bins = dead slots."""
    import collections

    buckets = collections.defaultdict(list)
    for e, z in enumerate(z_idx):
        buckets[int(z)].append(e)
    items = list(buckets.items())
    units = [[it] for it in items if len(it[1]) % SLOTS == 0]
    rn = sorted(
        (it for it in items if len(it[1]) % SLOTS != 0),
        key=lambda t: len(t[1]) % SLOTS,
    )
    lo, hi = 0, len(rn) - 1
    while lo < hi:
        if (len(rn[lo][1]) % SLOTS) + (len(rn[hi][1]) % SLOTS) <= SLOTS:
            units.append([rn[lo], rn[hi]])
            lo += 1
            hi -= 1
        else:
            units.append([rn[hi]])
            hi -= 1
    if lo == hi:
        units.append([rn[lo]])

    bins = []
    for unit in units:
        stream = [(z, e) for z, es in unit for e in es]
        for s0 in range(0, len(stream), SLOTS):
            chunk = stream[s0 : s0 + SLOTS]
            zs = []
            for z, _ in chunk:
                if z not in zs:
                    zs.append(z)
            assert len(zs) <= 2
            zA = zs[0]
            zB = zs[1] if len(zs) > 1 else zs[0]
            binE = [e for _, e in chunk]
            half = [0 if z == zA else 1 for z, _ in chunk]
            bins.append((binE, half, zA, zB))
    assert len(bins) <= NG, f"packing produced {len(bins)} bins > {NG}"
    while len(bins) < NG:
        bins.append(([], [], 0, 0))
    return bins


def _host_prep(xyz, n_photons, coeffs, inv_voxel_size, psf_center):
    u = xyz * inv_voxel_size  # (B,N,3)
    u = u.copy()
    u[..., :2] -= psf_center[:2]
    u[..., 2] += psf_center[2]
    u_floor = np.floor(u)
    frac = u - u_floor
    ui = u_floor.astype(np.int32)
    x_idx = ui[..., 0] + PW  # (B,N)
    y_idx = ui[..., 1] + PH
    z_idx = ui[..., 2]
    frac[..., :2] = 1.0 - frac[..., :2]

    # 64-term series: series[b,n,c], c = kz*16 + kx*4 + ky
    p = frac[..., None] ** np.arange(4, dtype=np.float32)  # (B,N,3,4)
    vx, vy, vz = p[..., 0, :], p[..., 1, :], p[..., 2, :]
    series = (
        vz[..., :, None, None] * vx[..., None, :, None] * vy[..., None, None, :]
    ).reshape(B, N, C)
    s16 = n_photons[..., None] * series[:, :, None, :]  # (B,N,F,C)

    coeffs_t = np.ascontiguousarray(
        coeffs.reshape(D, HW, C).transpose(0, 2, 1)
    ).astype(ml_dtypes.bfloat16)  # (D, C, HW)

    off_all = ((y_idx - ROW0) * CSTRIDE + (x_idx - ROW0)).astype(np.int32)

    grp = np.zeros((B, NG, 128, 128 + HW), dtype=ml_dtypes.bfloat16)
    lhsT = np.zeros((128, 128), dtype=np.float32)
    offs_v = np.zeros((B, 1, N_DVE + 1), dtype=np.int32)
    offs_g = np.zeros((B, 1, N_GP + 1), dtype=np.int32)

    for b in range(B):
        bins = _pack_bins(z_idx[b])
        nv = ng = 0
        for g, (binE, half, zA, zB) in enumerate(bins):
            grp[b, g, 0:64, 128:] = coeffs_t[zA]
            grp[b, g, 64:128, 128:] = coeffs_t[zB]
            # steer an even-x emitter into slot 0: GPSIMD ops then stay
            # 4B-aligned (DVE handles any parity via its doubled canvas).
            # Swap only within the zA block (half assignment preserved).
            if USE_GP_SCATTER and len(binE) > 1:
                for j in range(len(binE)):
                    if half[j] != 0:
                        break
                    if x_idx[b, binE[j]] % 2 == 0:
                        binE[0], binE[j] = binE[j], binE[0]
                        break
            lhsT[:] = 0.0
            for i in range(SLOTS):
                if i < len(binE):
                    e, h = binE[i], half[i]
                    lhsT[h * 64 : h * 64 + 64, i * SW : i * SW + F] = s16[b, e].T
                    off = int(off_all[b, e])
                    odd = (x_idx[b, e] - ROW0) % 2
                else:
                    off = 0  # dead slot: zero weights, scatter adds zeros
                    odd = 0
                if _gp_slot(g, i):
                    offs_g[b, 0, ng] = off
                    ng += 1
                else:
                    # odd-x -> doubled-canvas region B at an even offset
                    offs_v[b, 0, nv] = off if not odd else CV_ELEMS + off - 1
                    nv += 1
            grp[b, g, :, 0:128] = lhsT
        assert nv == N_DVE and ng == N_GP

    return grp, offs_v, offs_g


def make_in_maps(np_inputs):
    grp, offs_v, offs_g = _host_prep(
        np.asarray(np_inputs["xyz"], dtype=np.float32),
        np.asarray(np_inputs["n_photons"], dtype=np.float32),
        np.asarray(np_inputs["coeffs"], dtype=np.float32),
        np.asarray(np_inputs["inv_voxel_size"], dtype=np.float32),
        np.asarray(np_inputs["psf_center"], dtype=np.float32),
    )
    return [
        {
            "grp": grp[b],
            "offs_v": offs_v[b],
            "offs_g": offs_g[b],
        }
        for b in range(B)
    ]


def get_compiled():
    global _compiled
    if _compiled is None:
        _compiled = _build_bass()
    return _compiled


def kernel(xyz, n_photons, coeffs, inv_voxel_size, psf_center, img_size):
    in_maps = make_in_maps(
        {
            "xyz": xyz,
            "n_photons": n_photons,
            "coeffs": coeffs,
            "inv_voxel_size": inv_voxel_size,
            "psf_center": psf_center,
        }
    )
    nc = get_compiled()
    res = run_bass_kernel_spmd(nc, in_maps, core_ids=list(range(B)))
    out = np.stack(
        [
            res.results[b]["out"]
            .astype(np.float32)
            .reshape(3, F, IMG, IMG)
            .sum(axis=0)
            for b in range(B)
        ],
        axis=0,
    )
    return out
